# revision 59
# baseline (speedup 1.0000x reference)
"""Single-head causal attention (B=128, T=512, C=256, H=64) on 8 trn2 cores.

Data-parallel: 16 batches per core. x is transposed on the host to [B, C, T]
so the kernel loads xT directly (SWDGE fp32->bf16 cast, quad-batched): no
on-chip transpose, no psum->sbuf xT copy.

Per batch:
  [qT|kT] pair projections (batch j of a pair on partitions 64j..64j+63)
  simT[s,t] = kT-slice^T @ qT, all four s-chunks packed into one PSUM
  region (si0@0, si1@512, si3@896, si2@1024 floats; each matmul output
  stays inside one 2KB bank) so a single ACT instruction does exp over
  all 1280 columns. Causal diag masking accumulates -240 into the diag
  blocks via fp8e4 DoubleRow matmuls (half the cost of bf16 matmuls).
  v = xT^T @ Wv (natural [s,h] layout), ones column appended for rowsum.
  out_unnorm[t,h], rowsum[t] = pT^T @ [v|1];  out = out_unnorm * recip
  -> bf16 store; host upcasts to fp32.

Emission is software-pipelined with a one-batch skew: AV/normalize/store of
batch b-1 are emitted after sim of batch b so the PE does not block on exp.
"""
import numpy as np
import ml_dtypes

B, T, C, H = 128, 512, 256, 64
N_CORES = 8
BL = B // N_CORES          # batches per core
TC = T // 128              # 4 t-chunks
CS = C // 128              # 2 c-subtiles
INV_SQRT_H = 1.0 / np.sqrt(H)
QUAD = 4                   # batches per SWDGE load
# packed psim float offsets per s-chunk (each matmul output within one bank)
OFF = {0: 0, 1: 512, 2: 1024, 3: 896}
MASK_NEG = -240.0          # fp8e4 (trn e4m3) max normal


def _build_program():
    import concourse.tile as tile
    from concourse import bacc, mybir

    dt = mybir.dt
    DR = mybir.MatmulPerfMode.DoubleRow
    nc = bacc.Bacc("TRN2", target_bir_lowering=False, debug=False,
                   enable_asserts=False, num_devices=N_CORES)

    x_d = nc.dram_tensor("xt", [BL, C, T], dt.float32,
                         kind="ExternalInput").ap()
    wall_d = nc.dram_tensor("wall", [128, 3, CS, H], dt.bfloat16,
                            kind="ExternalInput").ap()
    msk_d = nc.dram_tensor("msk", [128, 2, 2, 128], dt.float8e4,
                           kind="ExternalInput").ap()
    # partition-major output layout: per-partition DRAM runs of TC*H*2 =
    # 512B (vs 128B for [T, H]) -> full DMA descriptor rate. Host unshuffles.
    out_d = nc.dram_tensor("out", [BL, 128, TC, H], dt.bfloat16,
                           kind="ExternalOutput").ap()

    with tile.TileContext(nc) as tc:
        from contextlib import ExitStack
        ctx = ExitStack()
        with ctx:
            consts = ctx.enter_context(tc.tile_pool(name="consts", bufs=1))
            sb_x = ctx.enter_context(tc.tile_pool(name="sb_x", bufs=2))
            sb_qk = ctx.enter_context(tc.tile_pool(name="sb_qk", bufs=4))
            sb_p = ctx.enter_context(tc.tile_pool(name="sb_p", bufs=4))
            sb_v = ctx.enter_context(tc.tile_pool(name="sb_v", bufs=4))
            sb_o = ctx.enter_context(tc.tile_pool(name="sb_o", bufs=8))
            ps_qk = ctx.enter_context(tc.tile_pool(name="ps_qk", bufs=1,
                                                   space="PSUM"))
            ps_sima = ctx.enter_context(tc.tile_pool(name="ps_sima", bufs=2,
                                                     space="PSUM"))
            ps_simb = ctx.enter_context(tc.tile_pool(name="ps_simb", bufs=1,
                                                     space="PSUM"))
            ps_simc = ctx.enter_context(tc.tile_pool(name="ps_simc", bufs=1,
                                                     space="PSUM"))
            ps_v = ctx.enter_context(tc.tile_pool(name="ps_v", bufs=1,
                                                  space="PSUM"))
            ps_av = ctx.enter_context(tc.tile_pool(name="ps_av", bufs=1,
                                                   space="PSUM"))

            wall_sb = consts.tile([128, 3, CS, H], dt.bfloat16)
            nc.sync.dma_start(wall_sb[:], wall_d)
            msk_sb = consts.tile([128, 2, 2, 128], dt.float8e4)
            nc.scalar.dma_start(msk_sb[:], msk_d)
            wq_sb = wall_sb[:, 0]
            wk_sb = wall_sb[:, 1]
            wv_sb = wall_sb[:, 2]
            i2_sb = msk_sb[:, 0]
            mn2_sb = msk_sb[:, 1]

            xt_tiles = {}
            qk_tiles = {}
            pT_tiles = {}
            v1_tiles = {}
            pav_tiles = {}

            def emit_load(b0, nb, split_cs=False):
                xtn = sb_x.tile([128, nb, CS, T], dt.bfloat16,
                                name=f"xt{b0}", tag=f"xt{nb}")
                if split_cs:
                    for cs in range(CS):
                        nc.gpsimd.dma_start(
                            xtn[:, :, cs], x_d[b0:b0 + nb, 128 * cs:
                                               128 * (cs + 1)].rearrange(
                                "b (cs p) t -> p b cs t", p=128)[:, :, 0])
                else:
                    nc.gpsimd.dma_start(
                        xtn[:], x_d[b0:b0 + nb].rearrange(
                            "b (cs p) t -> p b cs t", p=128))
                for j in range(nb):
                    xt_tiles[b0 + j] = xtn[:, j]

            def emit_proj(pb):
                # pair projections: batch j on psum partitions 64j..64j+63
                pq = ps_qk.tile([128, T], dt.float32, name=f"pq{pb}",
                                tag="pq")
                pk = ps_qk.tile([128, T], dt.float32, name=f"pk{pb}",
                                tag="pk")
                for j in range(2):
                    xt = xt_tiles[2 * pb + j]
                    for w_sb, pt in ((wq_sb, pq), (wk_sb, pk)):
                        for cc in range(CS):
                            nc.tensor.matmul(
                                pt[64 * j:64 * (j + 1), :],
                                w_sb[:, cc, :], xt[:, cc, :],
                                start=(cc == 0), stop=(cc == CS - 1),
                                tile_position=(0, 64 * j))
                qk = sb_qk.tile([128, 2, T], dt.bfloat16, name=f"qk{pb}",
                                tag="qk")
                nc.vector.tensor_copy(qk[:, 0, :], pq[:])
                nc.vector.tensor_copy(qk[:, 1, :], pk[:])
                qk_tiles[2 * pb] = (qk, True)
                qk_tiles[2 * pb + 1] = (qk, True)

            def emit_proj_solo(b):
                # pipeline fill: batches 0/1 get solo projections so batch 0's
                # sims don't wait for batch 1's load (pair copies couple them)
                pq = ps_qk.tile([128, T], dt.float32, name=f"pq_s{b}",
                                tag="pq")
                pk = ps_qk.tile([128, T], dt.float32, name=f"pk_s{b}",
                                tag="pk")
                xt = xt_tiles[b]
                for w_sb, pt in ((wq_sb, pq), (wk_sb, pk)):
                    for cc in range(CS):
                        nc.tensor.matmul(
                            pt[0:64, :], w_sb[:, cc, :], xt[:, cc, :],
                            start=(cc == 0), stop=(cc == CS - 1),
                            tile_position=(0, 0))
                qk = sb_qk.tile([64, 2, T], dt.bfloat16, name=f"qk_s{b}",
                                tag=f"qks{b}")
                nc.vector.tensor_copy(qk[:, 0, :], pq[0:64, :])
                nc.vector.tensor_copy(qk[:, 1, :], pk[0:64, :])
                qk_tiles[b] = (qk, False)

            def emit_v(b):
                xt = xt_tiles[b]
                pv = ps_v.tile([128, TC, 128], dt.float32, name=f"pv{b}",
                               tag="pv")
                for sc in range(TC):
                    for cc in range(CS):
                        nc.tensor.matmul(
                            pv[:, sc, 0:H],
                            xt[:, cc, 128 * sc:128 * (sc + 1)],
                            wv_sb[:, cc, :],
                            start=(cc == 0), stop=(cc == CS - 1))
                v1 = sb_v.tile([128, TC, H + 1], dt.bfloat16,
                               name=f"v1{b}", tag="v1")
                nc.vector.tensor_copy(v1[:, :, 0:H], pv[:, :, 0:H])
                nc.gpsimd.memset(v1[:, :, H:H + 1], 1.0)
                v1_tiles[b] = v1

            def emit_sim_exp(b):
                qk, paired = qk_tiles[b]
                j = (b % 2) if paired else 0
                qT = qk[64 * j:64 * (j + 1), 0, :]
                kT = qk[64 * j:64 * (j + 1), 1, :]
                # si0 in its own double-buffered tile so its exp can run
                # while si1/2/3 matmuls of this batch (and si0 of the next
                # batch) proceed; si1/si3/si2 pack into psimB at offsets
                # 0/384/512 (pT offsets 512/896/1024).
                psimA = ps_sima.tile([128, 512], dt.float32,
                                     name=f"psimA{b}", tag="psimA")
                psimB = ps_simb.tile([128, 512], dt.float32,
                                     name=f"psimB{b}", tag="psimB")
                psimC = ps_simc.tile([128, 512], dt.float32,
                                     name=f"psimC{b}", tag="psimC")
                pT = sb_p.tile([128, 1280], dt.bfloat16, name=f"pT{b}",
                               tag="pT")

                def sim_mask(si, reg, roff):
                    n_si = T - 128 * si
                    nc.tensor.matmul(
                        reg[:, roff:roff + n_si],
                        kT[:, 128 * si:128 * (si + 1)],
                        qT[:, 128 * si:T],
                        start=True, stop=False,
                        tile_position=(64 * j, 0),
                        skip_group_check=True)
                    nc.tensor.matmul(
                        reg[:, roff:roff + 128],
                        i2_sb[:], mn2_sb[:],
                        start=False, stop=True,
                        perf_mode=DR,
                        skip_group_check=True)

                sim_mask(0, psimA, 0)
                nc.scalar.activation(
                    pT[:, 0:512], psimA[:],
                    mybir.ActivationFunctionType.Exp,
                    scale=float(INV_SQRT_H))
                sim_mask(1, psimB, 0)
                sim_mask(3, psimB, 384)
                nc.scalar.activation(
                    pT[:, 512:1024], psimB[:],
                    mybir.ActivationFunctionType.Exp,
                    scale=float(INV_SQRT_H))
                sim_mask(2, psimC, 0)
                nc.scalar.activation(
                    pT[:, 1024:1280], psimC[:, 0:256],
                    mybir.ActivationFunctionType.Exp,
                    scale=float(INV_SQRT_H))
                pT_tiles[b] = pT

            def emit_av(b):
                pT = pT_tiles[b]
                v1 = v1_tiles[b]
                pav = ps_av.tile([128, TC, 128], dt.float32,
                                 name=f"pav{b}", tag="pav")
                for ci in range(TC):
                    for si in range(ci + 1):
                        nc.tensor.matmul(
                            pav[:, ci, 0:H + 1],
                            pT[:, OFF[si] + 128 * (ci - si):
                               OFF[si] + 128 * (ci - si) + 128],
                            v1[:, si, :],
                            start=(si == 0), stop=(si == ci))
                pav_tiles[b] = pav

            def emit_out(b):
                pav = pav_tiles[b]
                rec = sb_o.tile([128, TC], dt.float32, name=f"rec{b}",
                                tag="rec")
                nc.vector.reciprocal(rec[:], pav[:, :, H])
                osb = sb_o.tile([128, TC, H], dt.bfloat16,
                                name=f"osb{b}", tag="osb")
                nc.vector.tensor_mul(
                    out=osb[:],
                    in0=pav[:, :, 0:H],
                    in1=rec[:, :, None].to_broadcast([128, TC, H]))
                nc.sync.dma_start(out_d[b], osb[:])

            # software-pipelined emission: projections for the next pair are
            # emitted mid-pair (after sim of the odd batch) so their psum->
            # sbuf copies run a full batch before the sims that consume them;
            # AV/out of batch b-1 go after sim of b so PE never waits on exp.
            # first loads staggered (1,1,2) so proj(0) starts early;
            # steady state uses quad loads.
            emit_load(0, 1)
            emit_load(1, 1)
            emit_load(2, 2)
            emit_proj_solo(0)
            emit_proj_solo(1)
            for b in range(BL + 1):
                if b < BL:
                    if b % QUAD == 3 and b + 1 < BL:
                        emit_load(b + 1, QUAD)
                    emit_v(b)
                    emit_sim_exp(b)
                    if b % 2 == 1 and (b + 1) // 2 < BL // 2:
                        emit_proj((b + 1) // 2)
                if 1 <= b <= BL:
                    emit_av(b - 1)
                    emit_out(b - 1)

    nc.compile()
    return nc


_CACHED = None


def _get_program():
    global _CACHED
    if _CACHED is None:
        _CACHED = _build_program()
    return _CACHED


def _host_inputs(Wq, Wk, Wv):
    bf16 = ml_dtypes.bfloat16
    fp8 = ml_dtypes.float8_e4m3
    # wall[p, i, cs, h] = W_i[cs*128 + p, h]
    wall = np.stack([np.asarray(w, np.float32).reshape(CS, 128, H)
                     for w in (Wq, Wk, Wv)], axis=0)      # [3, CS, 128, H]
    wall = np.ascontiguousarray(wall.transpose(2, 0, 1, 3))  # [128, 3, CS, H]
    msk = np.zeros((128, 2, 2, 128), np.float32)
    msk[:, 0, 0, :] = np.eye(128, dtype=np.float32)
    msk[:, 1, 0, :] = np.where(
        np.arange(128)[None, :] < np.arange(128)[:, None],
        np.float32(MASK_NEG), np.float32(0))
    consts = {
        "wall": wall.astype(bf16),
        "msk": msk.astype(fp8),
    }
    return consts


def kernel(input_embeddings, Wq, Wk, Wv):
    from concourse.bass_utils import run_bass_kernel_spmd

    x = np.asarray(input_embeddings, np.float32)
    xt = np.ascontiguousarray(x.transpose(0, 2, 1))   # [B, C, T]
    nc = _get_program()
    consts = _host_inputs(Wq, Wk, Wv)
    in_maps = []
    for c in range(N_CORES):
        m = {"xt": xt[c * BL:(c + 1) * BL]}
        m.update(consts)
        in_maps.append(m)
    res = run_bass_kernel_spmd(nc, in_maps, core_ids=list(range(N_CORES)))
    out = np.concatenate([res.results[c]["out"] for c in range(N_CORES)],
                         axis=0)                    # [B, 128, TC, H]
    out = out.transpose(0, 2, 1, 3).reshape(B, T, H)  # t = tc*128 + p
    return np.ascontiguousarray(out).astype(np.float32)


if __name__ == "__main__":
    rng = np.random.default_rng(0)
    x = rng.standard_normal((B, T, C)).astype(np.float32)
    wq = (rng.standard_normal((C, H)) / 16).astype(np.float32)
    wk = (rng.standard_normal((C, H)) / 16).astype(np.float32)
    wv = (rng.standard_normal((C, H)) / 16).astype(np.float32)
    out = kernel(x, wq, wk, wv)
    print("out", out.shape, out.dtype)
